# revision 47
# baseline (speedup 1.0000x reference)
"""Multi-head attention forward on 8 Trainium2 NeuronCores (Bass/Tile), v3.

Problem: nn_MultiHeadAttention — B=8, T=1024, C=768, H=12, D=64, fp32 in/out.

Sharding: data-parallel over batch — one batch element per core; weights
broadcast (each core loads its own copy). No collectives. Host pre-transposes
x[b] to x^T [C, T] and converts x/W to bf16; fp32 PSUM accumulation keeps the
final rel err ~6.4e-3 (tolerance 2e-2).

Per-core kernel, all matmul operands bf16:
  1. V = x @ Wv -> V_aug [128, T/128, H, 65] bf16 with a ones column per head
     (the ones row of att@V yields the softmax denominator for free).
  2. Per head pair p: Q^T/K^T chunks [128, T] (pair-major: head A on
     partitions 0-63, head B on 64-127) via matmul(lhsT=W[:,co], rhs=xT).
     Pair 0 projects into the idle SA/SB PSUM regions (no aux serialization).
     The mid-loop K projections evacuate cols [0:512] first and the head-B
     normalize mult sits at j=7, so the next pair's S^T never queues behind
     a full-width DVE evacuation at the boundary.
  3. S^T per pair via ROW-TILED matmuls (tile_position (0,0)/(64,0) run the
     two heads concurrently on the PE), SPLIT per head into two separate
     2-bank PSUM regions (SA for head A, SB for head B) with one exp each:
     the PE can start S_A(j+1) as soon as exp_A(j) is done, while exp_B(j)
     still runs (v2's fused [128,2048] region serialized PE<->ACT per j).
     At j=7 the exps are split into 512-col halves (B first on the last
     pair) so pair-boundary/tail consumers wake earlier.
  4. att@V: Ytil[65, :] += V_aug^T @ pt chunks, software-pipelined one j
     behind the exp. Head B's att@V runs early in the NEXT pair's j-loop
     (js 0-2); next pair's Q/K projections on js 3-6; the last pair's B runs
     inline on js 3-7. PSUM: SA 2 + SB 2 + psyA 2 + aux(time-shared) 2 = 8
     banks exactly.
  5. Normalization at Ytil evacuation: denominator row staged via a DVE
     cross-base copy to partition 0 (the custom DVE reciprocal silently
     misreads any other base; ScalarE must NOT do this copy — it sits on
     the cycle-critical exp queue, HW-measured ~7us slower), DVE
     reciprocal, GpSimd partition-broadcast (~1.5us, off the PE), DVE copy
     of the Ytil rows (frees the accumulator banks), DVE multiply into YT.
  6. Tail/output projection: the normalize chain of the last pair hides
     behind out-proj k=0..4 partial-sum matmuls for the first 4 t-chunks on
     all four freed PSUM regions; only the k=5 matmuls wait. 4 rotating ot
     buffers so the last bias-adds don't wait on earlier output DMAs
     (scalar-engine ring).
  Prologue: xT[:, :128] + Wv load first (sync ring) so the first V-proj
  matmul issues at ~4us; remaining xT t-chunks stream on the GpSimd SWDGE
  ring concurrently with the other weights (their reps-loop WAR releases
  late and would block the sync ring). Wp loads into its own buffer.

HW notes (learned the hard way, enforced by CoreSim/BIR but not TimelineSim):
  - PSUM matmul accumulation groups must be bank-aligned (512 fp32); a
    matmul may not cross a bank boundary.
  - Engine APs touching PSUM need 32-aligned partition bases.
  - Finer-grained pipelining (4 exps/j with alternating 1-bank S tiles)
    measured SLOWER on HW: per-hop semaphore latency dominates, so fewer,
    bigger sync units win.
  - Attention steady state is bound by the serial cycle S_A(j) -> exp_A(j)
    -> S_A(j+1) (~3.4us/j at exp=764ns); deeper S buffering needs 12 PSUM
    banks (only 8 exist). Keep the ACT queue clean: exps only.
"""
import numpy as np

B, T, C = 8, 1024, 768
H, D = 12, 64
P = 128
KS = C // P          # 6 contraction subtiles
TS = T // P          # 8 t subtiles
NP = H // 2          # 6 head pairs
N_CORES = 8

_RUNNER_CACHE = {}


def build_nc(reps: int = 1, phases: int = 4, npairs: int = NP):
    import concourse.bacc as bacc
    import concourse.mybir as mybir
    import concourse.tile as tile
    from contextlib import ExitStack

    f32 = mybir.dt.float32
    bf16 = mybir.dt.bfloat16
    AF = mybir.ActivationFunctionType
    ALU = mybir.AluOpType

    nc = bacc.Bacc(num_devices=N_CORES)

    xT_d = nc.dram_tensor("xT", [C, T], bf16, kind="ExternalInput")
    W_d = {w: nc.dram_tensor(f"W{w}", [C, C], bf16, kind="ExternalInput")
           for w in ("q", "k", "v", "p")}
    bqT_d = nc.dram_tensor("bqT", [P, KS], f32, kind="ExternalInput")
    bkT_d = nc.dram_tensor("bkT", [P, KS], f32, kind="ExternalInput")
    bvB_d = nc.dram_tensor("bvB", [P, C], f32, kind="ExternalInput")
    bpB_d = nc.dram_tensor("bpB", [P, C], f32, kind="ExternalInput")
    y_d = nc.dram_tensor("y", [T, C], f32, kind="ExternalOutput")

    with tile.TileContext(nc) as tc, ExitStack() as ctx:
        const = ctx.enter_context(tc.tile_pool(name="const", bufs=1))
        qkp = ctx.enter_context(tc.tile_pool(name="qk", bufs=1))
        ptAp = ctx.enter_context(tc.tile_pool(name="ptA", bufs=1))
        ptBp = ctx.enter_context(tc.tile_pool(name="ptB", bufs=1))
        opool = ctx.enter_context(tc.tile_pool(name="out", bufs=1))
        psSA = ctx.enter_context(tc.tile_pool(name="psSA", bufs=1, space="PSUM"))
        psSB = ctx.enter_context(tc.tile_pool(name="psSB", bufs=1, space="PSUM"))
        psY = ctx.enter_context(tc.tile_pool(name="psY", bufs=1, space="PSUM"))
        psX = ctx.enter_context(tc.tile_pool(name="psX", bufs=1, space="PSUM"))

        def body(_iv=None):
            # ---- loads ----
            # Chunked so the first V-proj matmul only waits for Wv[:, :512]
            # and xT[:, :256] (~3.5us of DMA) rather than every input. xT
            # goes on the DVE ring: at the reps-loop boundary its WAR (the
            # last pair's K projection) releases late, and on a shared ring
            # it would block the weight reloads queued behind it.
            xTr = const.tile([P, KS, T], bf16, tag="xT", name="xTr")
            Wr = {}
            for w in ("q", "k", "v", "p"):
                Wr[w] = const.tile([P, KS, C], bf16, tag=f"W{w}", name=f"W{w}r")
            xT_r = xT_d.rearrange("(ks p) t -> p ks t", p=P)
            W_r = {w: W_d[w].rearrange("(ks p) c -> p ks c", p=P)
                   for w in ("q", "k", "v", "p")}
            nc.sync.dma_start(xTr[:, :, 0:128], xT_r[:, :, 0:128])
            nc.sync.dma_start(Wr["v"][:, :, 0:512], W_r["v"][:, :, 0:512])
            nc.sync.dma_start(Wr["v"][:, :, 512:768], W_r["v"][:, :, 512:768])
            bvB = const.tile([P, C], f32, tag="bvB", name="bvB")
            nc.sync.dma_start(bvB[:], bvB_d[:, :])
            nc.gpsimd.dma_start(xTr[:, :, 128:384], xT_r[:, :, 128:384])
            nc.gpsimd.dma_start(xTr[:, :, 384:768], xT_r[:, :, 384:768])
            nc.gpsimd.dma_start(xTr[:, :, 768:1024], xT_r[:, :, 768:1024])
            nc.sync.dma_start(Wr["q"][:], W_r["q"])
            nc.sync.dma_start(Wr["k"][:], W_r["k"])
            bqT = const.tile([P, KS], f32, tag="bqT", name="bqT")
            nc.sync.dma_start(bqT[:], bqT_d[:, :])
            bkT = const.tile([P, KS], f32, tag="bkT", name="bkT")
            nc.sync.dma_start(bkT[:], bkT_d[:, :])
            nc.sync.dma_start(Wr["p"][:], W_r["p"])
            bpB = const.tile([P, C], f32, tag="bpB", name="bpB")
            nc.sync.dma_start(bpB[:], bpB_d[:, :])

            ones1 = const.tile([P, 1], f32, tag="ones", name="ones1")
            nc.vector.memset(ones1[:], 1.0)

            # ---- V projection into V_aug (bf16) with ones column ----
            # ones at column 64: att@V's denominator row lands on PSUM
            # partition 64 (32-aligned, so the DVE reciprocal can read it
            # directly from PSUM — no ScalarE staging copy).
            V_aug = const.tile([P, TS, H, D + 1], bf16, tag="Vaug", name="Vaug")
            nc.vector.tensor_copy(V_aug[:, :, :, D:D + 1],
                                  ones1[:].to_broadcast([P, TS, H, 1]))
            for ts_ in range(TS):
                psv = (psY.tile([P, 1024], f32, tag="psyA", name="psv")
                       if ts_ % 2 == 0 else
                       psX.tile([P, 1024], f32, tag="aux", name="psv"))
                # NOTE: matmul groups must stay PSUM-bank-aligned (512
                # fp32): sub-bank column splits create two accumulation
                # groups in one bank, which corrupts silently on HW.
                for k in range(KS):
                    lhsT = xTr[:, k, ts_ * P:(ts_ + 1) * P]
                    nc.tensor.matmul(psv[:, 0:512], lhsT, Wr["v"][:, k, 0:512],
                                     start=(k == 0), stop=(k == KS - 1))
                    nc.tensor.matmul(psv[:, 512:768], lhsT, Wr["v"][:, k, 512:768],
                                     start=(k == 0), stop=(k == KS - 1))
                nc.vector.tensor_tensor(
                    V_aug[:, ts_, :, 0:D],
                    psv[:, 0:768].rearrange("p (h d) -> p h d", h=H),
                    bvB[:].rearrange("p (h d) -> p h d", h=H), op=ALU.add)

            if phases < 3:
                YTdummy = opool.tile([P, C], f32, tag="ot0", name="ytd")
                nc.vector.memset(YTdummy[:], 0.0)
                nc.sync.dma_start(y_d[0:P, :], YTdummy[:])
                return

            YT = const.tile([P, KS, T], bf16, tag="YTs", name="YT")

            def emit_qk_mms(p, which, ps, kslice):
                w = "q" if which == "Q" else "k"
                for ih in range(2):
                    for k in kslice:
                        nc.tensor.matmul(
                            ps[:, ih * 512:(ih + 1) * 512],
                            Wr[w][:, k, p * P:(p + 1) * P],
                            xTr[:, k, ih * 512:(ih + 1) * 512],
                            start=(k == 0), stop=(k == KS - 1))

            def emit_qk_evac(p, which, ps, split=False):
                bias = bqT if which == "Q" else bkT
                out = qkp.tile([P, T], bf16, tag=f"{which}{p % 2}", name=f"{which}T2")
                # split=True: evacuate cols [0:512] first — the next pair's
                # S^T js 0-3 read only that half (subtile deps), so it can
                # start while the second half still evacuates
                for c0, c1 in ((0, 512), (512, 1024)) if split else ((0, 1024),):
                    nc.vector.tensor_tensor(
                        out[:, c0:c1], ps[:, c0:c1],
                        bias[:, p:p + 1].to_broadcast([P, c1 - c0]),
                        op=ALU.add)
                return out

            def emit_qk_proj(p, which):
                # pair-0 only: SA/SB are idle here, and using them avoids
                # serializing Q-proj -> aux evac -> K-proj -> aux evac
                ps = (psSA.tile([P, 1024], f32, tag="SA", name=f"ps{which}")
                      if which == "Q" else
                      psSB.tile([P, 1024], f32, tag="SB", name=f"ps{which}"))
                emit_qk_mms(p, which, ps, range(KS))
                # split evac: S^T(0,0) writes these regions and can start
                # once cols [0:512] are evacuated
                return emit_qk_evac(p, which, ps, split=True)

            def evac_copy(p, hh, psy):
                """Fast PSUM->SBUF evacuation of the Ytil rows via DVE.
                The denominator row (64) is read by evac_recip directly."""
                tmp = qkp.tile([P, T], f32, tag=f"tmp{hh}{p % 2}", name="tmp")
                nc.vector.tensor_copy(tmp[0:D, :], psy[0:D, :])
                return tmp

            def evac_recip(p, hh, psy):
                """Reciprocal of the denominator row (read straight from PSUM
                partition 0) + GpSimd partition-broadcast across 64 rows; the
                ~1.5us broadcast runs while the PE continues."""
                h = 2 * p + hh
                # staging on DVE, NOT ScalarE: ACT sits on the cycle-
                # critical exp path (exp_A(j) gates S_A(j+1)), and a ~1us
                # copy at the head of its queue delays the whole pair.
                # DVE cross-base copy is legal (both bases 32-aligned).
                dst = qkp.tile([1, T], f32, tag=f"dst{h % 2}", name="dstage")
                nc.vector.tensor_copy(dst[:], psy[D:D + 1, :])
                rcp = qkp.tile([1, T], f32, tag=f"rcp{h % 2}", name="rcp")
                nc.vector.reciprocal_approx_fast(rcp[:], dst[:])
                rb = qkp.tile([D, T], f32, tag=f"rb{h % 2}", name="rb")
                nc.gpsimd.partition_broadcast(rb[:], rcp[:])
                return rb

            def evac_mult(p, hh, tmp, rb):
                b0 = 64 * hh
                nc.vector.tensor_tensor(YT[b0:b0 + 64, p, :], tmp[0:D, :],
                                        rb[:], op=ALU.mult)

            def attv(psy, v_slice, pt, j):
                for ih in range(2):
                    sl = slice(ih * 512, (ih + 1) * 512)
                    nc.tensor.matmul(psy[0:D + 1, sl], v_slice, pt[:, sl],
                                     start=(j == 0), stop=(j == TS - 1))

            # ---- attention pair loop (software-pipelined) ----
            qt_cur = emit_qk_proj(0, "Q")
            kt_cur = emit_qk_proj(0, "K")
            tmpA_prev = tmpB_prev = None
            psyA_prev = None
            qt_nxt = kt_nxt = None
            ptA_tiles = [None] * TS
            ptB_tiles = [None] * TS
            ptB_prev = [None] * TS

            for p in range(npairs):
                last = (p == NP - 1)
                rbA = None
                if psyA_prev is not None:
                    rbA = evac_recip(p - 1, 0, psyA_prev)
                    tmpA_prev = evac_copy(p - 1, 0, psyA_prev)
                psyA = psY.tile([P, 1024], f32, tag="psyA", name="psyA")

                prevB = None
                if p > 0:
                    prevB = psX.tile([P, 1024], f32, tag="aux", name="psyBprev")
                psyB = None

                qps = kps = None
                for j in range(TS):
                    psA = psSA.tile([P, 1024], f32, tag="SA", name="psA")
                    psB = psSB.tile([P, 1024], f32, tag="SB", name="psB")
                    ptA = ptAp.tile([P, 1024], bf16, tag=f"ptA{j}", name="ptA")
                    ptB = ptBp.tile([P, 1024], bf16, tag=f"ptB{j}", name="ptB")

                    # INTERLEAVED A/B emission (A-h0, B-h0, A-h1, B-h1): the
                    # two PE row-groups stream concurrently, so B-h0 starts
                    # while A-h1 streams; A,A,B,B order serializes the same-
                    # group pairs first and delays exp_B by two matmuls
                    for ih in range(2):
                        sl = slice(ih * 512, (ih + 1) * 512)
                        nc.tensor.matmul(psA[:, sl],
                                         kt_cur[0:64, j * P:(j + 1) * P],
                                         qt_cur[0:64, sl],
                                         start=True, stop=True,
                                         tile_position=(0, 0))
                        nc.tensor.matmul(psB[:, sl],
                                         kt_cur[64:128, j * P:(j + 1) * P],
                                         qt_cur[64:128, sl],
                                         start=True, stop=True,
                                         tile_position=(64, 0))
                    if j == TS - 1 and last:
                        # last pair: B's chain is the tail-critical one
                        # (inline attv_B(7) comes first) — emit its exp
                        # first, both split for finer consumer wake-up
                        for c0, c1 in ((0, 512), (512, 1024)):
                            nc.scalar.activation(ptB[:, c0:c1], psB[:, c0:c1],
                                                 AF.Exp, scale=0.125)
                        for c0, c1 in ((0, 512), (512, 1024)):
                            nc.scalar.activation(ptA[:, c0:c1], psA[:, c0:c1],
                                                 AF.Exp, scale=0.125)
                    elif j == TS - 1:
                        # split so the post-loop attv_A(7) (and the next
                        # pair's S^T) wait on a 512-col exp, not the full one
                        nc.scalar.activation(ptA[:, 0:512], psA[:, 0:512],
                                             AF.Exp, scale=0.125)
                        nc.scalar.activation(ptA[:, 512:1024], psA[:, 512:1024],
                                             AF.Exp, scale=0.125)
                        nc.scalar.activation(ptB[:], psB[:], AF.Exp, scale=0.125)
                    else:
                        nc.scalar.activation(ptA[:], psA[:], AF.Exp, scale=0.125)
                        nc.scalar.activation(ptB[:], psB[:], AF.Exp, scale=0.125)
                    ptA_tiles[j] = ptA
                    ptB_tiles[j] = ptB

                    # attV_A one j behind: exp_A(j-1) finished during S(j)
                    if j >= 1:
                        attv(psyA, V_aug[:, j - 1, 2 * p, :], ptA_tiles[j - 1], j - 1)
                    # previous pair's head-B att@V: 3/3/2 chunks on js 0-2,
                    # so its PSUM slot frees early for the next projections
                    if prevB is not None and j < 3:
                        for jj in range(3 * j, min(3 * j + 3, TS)):
                            attv(prevB, V_aug[:, jj, 2 * (p - 1) + 1, :],
                                 ptB_prev[jj], jj)
                        if j == 2:
                            rbB = evac_recip(p - 1, 1, prevB)
                            tmpB_prev = evac_copy(p - 1, 1, prevB)
                    if j == 3 and rbA is not None:
                        evac_mult(p - 1, 0, tmpA_prev, rbA)
                        tmpA_prev = rbA = None
                    if j == 7 and prevB is not None:
                        # j==7 (not 6): keeps the DVE free at j==6 so the kt
                        # evacuation isn't queued behind this mult — YT[p-1]
                        # is only read by the final out-projection anyway
                        evac_mult(p - 1, 1, tmpB_prev, rbB)
                    # next pair's projections on js 3-6
                    if p + 1 < NP:
                        if j == 3:
                            qps = psX.tile([P, 1024], f32, tag="aux", name="psQ")
                            emit_qk_mms(p + 1, "Q", qps, range(0, 3))
                        elif j == 4:
                            emit_qk_mms(p + 1, "Q", qps, range(3, KS))
                            qt_nxt = emit_qk_evac(p + 1, "Q", qps)
                        elif j == 5:
                            kps = psX.tile([P, 1024], f32, tag="aux", name="psK")
                            emit_qk_mms(p + 1, "K", kps, range(0, 3))
                        elif j == 6:
                            emit_qk_mms(p + 1, "K", kps, range(3, KS))
                            kt_nxt = emit_qk_evac(p + 1, "K", kps, split=True)
                    elif last:
                        # pair 5: head-B att@V inline on js 3-7 once the aux
                        # slot frees (accumulation order over j is free)
                        if j == 3:
                            psyB = psX.tile([P, 1024], f32, tag="aux", name="psyB5")
                        if j >= 3:
                            for jj in ([2 * (j - 3), 2 * (j - 3) + 1] if j <= 5
                                       else [j]):
                                attv(psyB, V_aug[:, jj, 2 * p + 1, :],
                                     ptB_tiles[jj], jj)

                attv(psyA, V_aug[:, TS - 1, 2 * p, :], ptA_tiles[TS - 1], TS - 1)
                psyA_prev = psyA
                ptB_prev = list(ptB_tiles)
                qt_cur, kt_cur = qt_nxt, kt_nxt

            # tail: last pair's evacuations, B chain first (its attv(7) is
            # inline and finishes before attv_A(7)). Reciprocals before
            # copies on the DVE so the GpSimd broadcasts launch early; the
            # normalize chain then hides behind the out-projection's k=0..4
            # partial sums on all four freed PSUM regions.
            rbB = tmpB_prev = None
            if psyB is not None:
                rbB = evac_recip(NP - 1, 1, psyB)
            rbA = evac_recip(npairs - 1, 0, psyA_prev)
            if psyB is not None:
                tmpB_prev = evac_copy(NP - 1, 1, psyB)
            tmpA_prev = evac_copy(npairs - 1, 0, psyA_prev)

            if phases < 4:
                # still need the normalize so YT is complete
                evac_mult(npairs - 1, 0, tmpA_prev, rbA)
                if psyB is not None:
                    evac_mult(NP - 1, 1, tmpB_prev, rbB)
                return

            # ---- output projection ----
            # psyA/aux host the EARLY chunks: the next rep's V projection
            # reuses those regions first, so their last readers (bias-adds)
            # must not be the final chunks of this rep
            def po_tile(ts_):
                pool, tag = ((psY, "psyA"), (psX, "aux"),
                             (psSA, "SA"), (psSB, "SB"))[ts_ % 4]
                return pool.tile([P, 1024], f32, tag=tag, name="po")

            def po_mms(po, ts_, kslice):
                for k in kslice:
                    lhsT = YT[:, k, ts_ * P:(ts_ + 1) * P]
                    nc.tensor.matmul(po[:, 0:512], lhsT, Wr["p"][:, k, 0:512],
                                     start=(k == 0), stop=(k == KS - 1))
                    nc.tensor.matmul(po[:, 512:768], lhsT, Wr["p"][:, k, 512:768],
                                     start=(k == 0), stop=(k == KS - 1))

            pos = {}
            for ts_ in (0, 1, 2, 3):
                pos[ts_] = po_tile(ts_)
                po_mms(pos[ts_], ts_, range(KS - 1))

            if psyB is not None:
                evac_mult(NP - 1, 1, tmpB_prev, rbB)
            evac_mult(npairs - 1, 0, tmpA_prev, rbA)

            for ts_ in range(TS):
                if ts_ in pos:
                    po = pos[ts_]
                    po_mms(po, ts_, range(KS - 1, KS))
                else:
                    po = po_tile(ts_)
                    po_mms(po, ts_, range(KS))
                ot = opool.tile([P, C], f32, tag=f"ot{ts_ % 4}", name="ot")
                nc.vector.tensor_tensor(ot[:], po[:, 0:768], bpB[:], op=ALU.add)
                nc.scalar.dma_start(y_d[ts_ * P:(ts_ + 1) * P, :], ot[:])

        if reps == 1:
            body()
        else:
            import concourse.mybir as _mb
            with tc.For_i(0, reps, 1, hint_engines=tuple(_mb.ALL_ENGINES)):
                body()

    nc.compile()
    return nc


class _Runner:
    """Compile once, run many times on the 8 axon-tunneled cores via PJRT."""

    def __init__(self, nc, n_cores):
        import jax
        import concourse.mybir as mybir
        from jax.sharding import Mesh, PartitionSpec
        from jax.experimental.shard_map import shard_map
        from concourse.bass2jax import (
            _bass_exec_p, install_neuronx_cc_hook, partition_id_tensor)

        install_neuronx_cc_hook()
        self.jax = jax
        self.n_cores = n_cores
        partition_name = nc.partition_id_tensor.name if nc.partition_id_tensor else None
        in_names, out_names, out_avals, zero_outs = [], [], [], []
        for alloc in nc.m.functions[0].allocations:
            if not isinstance(alloc, mybir.MemoryLocationSet):
                continue
            name = alloc.memorylocations[0].name
            if alloc.kind == "ExternalInput":
                if name != partition_name:
                    in_names.append(name)
            elif alloc.kind == "ExternalOutput":
                shape = tuple(alloc.tensor_shape)
                dtype = mybir.dt.np(alloc.dtype)
                out_names.append(name)
                out_avals.append(jax.core.ShapedArray(shape, dtype))
                zero_outs.append(np.zeros(shape, dtype))
        self.in_names, self.out_names = in_names, out_names
        self.zero_outs = zero_outs
        all_in = list(in_names) + list(out_names)
        if partition_name is not None:
            all_in.append(partition_name)

        def _body(*args):
            operands = list(args)
            if partition_name is not None:
                operands.append(partition_id_tensor())
            return tuple(_bass_exec_p.bind(
                *operands, out_avals=tuple(out_avals), in_names=tuple(all_in),
                out_names=tuple(out_names), lowering_input_output_aliases=(),
                sim_require_finite=True, sim_require_nnan=True, nc=nc))

        devices = jax.devices()[:n_cores]
        self.mesh = Mesh(np.asarray(devices), ("core",))
        spec = PartitionSpec("core")
        self.fn = jax.jit(
            shard_map(_body, mesh=self.mesh,
                      in_specs=(spec,) * (len(in_names) + len(out_names)),
                      out_specs=(spec,) * len(out_names), check_rep=False),
            keep_unused=True)

    def stage(self, in_maps):
        import jax
        from jax.sharding import PartitionSpec
        concat = [
            np.concatenate([np.asarray(in_maps[c][n]) for c in range(self.n_cores)], axis=0)
            for n in self.in_names
        ] + [np.concatenate([z] * self.n_cores, axis=0) for z in self.zero_outs]
        sharding = jax.sharding.NamedSharding(self.mesh, PartitionSpec("core"))
        return [jax.device_put(a, sharding) for a in concat]

    def run(self, staged):
        outs = self.fn(*staged)
        self.jax.block_until_ready(outs)
        return outs

    def run_to_maps(self, staged):
        outs = self.run(staged)
        res = []
        for c in range(self.n_cores):
            m = {}
            for i, n in enumerate(self.out_names):
                g = np.asarray(outs[i])
                per = g.shape[0] // self.n_cores
                m[n] = g[c * per:(c + 1) * per]
            res.append(m)
        return res


def get_runner(reps: int = 1, phases: int = 4, npairs: int = NP):
    key = (reps, phases, npairs)
    if key not in _RUNNER_CACHE:
        nc = build_nc(reps, phases, npairs)
        _RUNNER_CACHE[key] = _Runner(nc, N_CORES)
    return _RUNNER_CACHE[key]


def make_in_maps(x, Wq, bq, Wk, bk, Wv, bv, Wp, bp):
    import ml_dtypes
    bf = ml_dtypes.bfloat16
    x = np.asarray(x, dtype=np.float32)
    weights = {
        "Wq": np.asarray(Wq, bf), "Wk": np.asarray(Wk, bf),
        "Wv": np.asarray(Wv, bf), "Wp": np.asarray(Wp, bf),
    }
    bqT = np.ascontiguousarray(np.asarray(bq, np.float32).reshape(KS, P).T)
    bkT = np.ascontiguousarray(np.asarray(bk, np.float32).reshape(KS, P).T)
    bvB = np.ascontiguousarray(np.broadcast_to(np.asarray(bv, np.float32), (P, C)))
    bpB = np.ascontiguousarray(np.broadcast_to(np.asarray(bp, np.float32), (P, C)))
    in_maps = []
    for b in range(B):
        in_maps.append({
            "xT": np.ascontiguousarray(x[b].T).astype(bf),
            "Wq": weights["Wq"], "Wk": weights["Wk"],
            "Wv": weights["Wv"], "Wp": weights["Wp"],
            "bqT": bqT, "bkT": bkT, "bvB": bvB, "bpB": bpB,
        })
    return in_maps


def kernel(x, Wq, bq, Wk, bk, Wv, bv, Wp, bp):
    runner = get_runner(reps=1)
    in_maps = make_in_maps(x, Wq, bq, Wk, bk, Wv, bv, Wp, bp)
    staged = runner.stage(in_maps)
    res = runner.run_to_maps(staged)
    return np.stack([res[b]["y"] for b in range(B)], axis=0)


# revision 49
# speedup vs baseline: 1.0207x; 1.0207x over previous
"""Multi-head attention forward on 8 Trainium2 NeuronCores (Bass/Tile), v3.

Problem: nn_MultiHeadAttention — B=8, T=1024, C=768, H=12, D=64, fp32 in/out.

Sharding: data-parallel over batch — one batch element per core; weights
broadcast (each core loads its own copy). No collectives. Host pre-transposes
x[b] to x^T [C, T] and converts x/W to bf16; fp32 PSUM accumulation keeps the
final rel err ~6.4e-3 (tolerance 2e-2).

Per-core kernel, all matmul operands bf16:
  1. V = x @ Wv -> V_aug [128, T/128, H, 65] bf16 with a ones column per head
     (the ones row of att@V yields the softmax denominator for free).
  2. Per head pair p: Q^T/K^T chunks [128, T] (pair-major: head A on
     partitions 0-63, head B on 64-127) via matmul(lhsT=W[:,co], rhs=xT).
     Pair 0 projects into the idle SA/SB PSUM regions (no aux serialization).
     The mid-loop K projections evacuate cols [0:512] first and the head-B
     normalize mult sits at j=7, so the next pair's S^T never queues behind
     a full-width DVE evacuation at the boundary.
  3. S^T per pair via ROW-TILED matmuls (tile_position (0,0)/(64,0) run the
     two heads concurrently on the PE), SPLIT per head into two separate
     2-bank PSUM regions (SA for head A, SB for head B) with one exp each:
     the PE can start S_A(j+1) as soon as exp_A(j) is done, while exp_B(j)
     still runs (v2's fused [128,2048] region serialized PE<->ACT per j).
     At j=7 the exps are split into 512-col halves (B first on the last
     pair) so pair-boundary/tail consumers wake earlier.
  4. att@V: Ytil[65, :] += V_aug^T @ pt chunks, software-pipelined one j
     behind the exp. Head B's att@V runs early in the NEXT pair's j-loop
     (js 0-2); next pair's Q/K projections on js 3-6; the last pair's B runs
     inline on js 3-7. PSUM: SA 2 + SB 2 + psyA 2 + aux(time-shared) 2 = 8
     banks exactly.
  5. Normalization at Ytil evacuation: denominator row staged via a DVE
     cross-base copy to partition 0 (the custom DVE reciprocal silently
     misreads any other base; ScalarE must NOT do this copy — it sits on
     the cycle-critical exp queue, HW-measured ~7us slower), DVE
     reciprocal, GpSimd partition-broadcast (~1.5us, off the PE), DVE copy
     of the Ytil rows (frees the accumulator banks), DVE multiply into YT.
  6. Tail/output projection: the normalize chain of the last pair hides
     behind out-proj k=0..4 partial-sum matmuls for the first 4 t-chunks on
     all four freed PSUM regions; only the k=5 matmuls wait. 4 rotating ot
     buffers so the last bias-adds don't wait on earlier output DMAs
     (scalar-engine ring).
  Prologue: xT[:, :128] + Wv load first (sync ring) so the first V-proj
  matmul issues at ~4us; remaining xT t-chunks stream on the GpSimd SWDGE
  ring concurrently with the other weights (their reps-loop WAR releases
  late and would block the sync ring). Wp loads into its own buffer.

HW notes (learned the hard way, enforced by CoreSim/BIR but not TimelineSim):
  - PSUM matmul accumulation groups must be bank-aligned (512 fp32); a
    matmul may not cross a bank boundary.
  - Engine APs touching PSUM need 32-aligned partition bases.
  - Finer-grained pipelining (4 exps/j with alternating 1-bank S tiles)
    measured SLOWER on HW: per-hop semaphore latency dominates, so fewer,
    bigger sync units win.
  - Attention steady state is bound by the serial cycle S_A(j) -> exp_A(j)
    -> S_A(j+1) (~3.4us/j at exp=764ns); deeper S buffering needs 12 PSUM
    banks (only 8 exist). Keep the ACT queue clean: exps only.
"""
import numpy as np

B, T, C = 8, 1024, 768
H, D = 12, 64
P = 128
KS = C // P          # 6 contraction subtiles
TS = T // P          # 8 t subtiles
NP = H // 2          # 6 head pairs
N_CORES = 8

_RUNNER_CACHE = {}


def build_nc(reps: int = 1, phases: int = 4, npairs: int = NP):
    import concourse.bacc as bacc
    import concourse.mybir as mybir
    import concourse.tile as tile
    from contextlib import ExitStack

    f32 = mybir.dt.float32
    bf16 = mybir.dt.bfloat16
    AF = mybir.ActivationFunctionType
    ALU = mybir.AluOpType

    nc = bacc.Bacc(num_devices=N_CORES)

    xT_d = nc.dram_tensor("xT", [C, T], bf16, kind="ExternalInput")
    W_d = {w: nc.dram_tensor(f"W{w}", [C, C], bf16, kind="ExternalInput")
           for w in ("v", "p")}
    # Wq|Wk packed into one tensor, all four biases into another: fewer
    # HWDGE descriptors (~625ns issue + latency gap each) on the sync ring
    Wqk_d = nc.dram_tensor("Wqk", [2 * C, C], bf16, kind="ExternalInput")
    bias_d = nc.dram_tensor("biases", [P, 2 * KS + 2 * C], f32,
                            kind="ExternalInput")
    y_d = nc.dram_tensor("y", [T, C], f32, kind="ExternalOutput")

    with tile.TileContext(nc) as tc, ExitStack() as ctx:
        const = ctx.enter_context(tc.tile_pool(name="const", bufs=1))
        qkp = ctx.enter_context(tc.tile_pool(name="qk", bufs=1))
        ptAp = ctx.enter_context(tc.tile_pool(name="ptA", bufs=1))
        ptBp = ctx.enter_context(tc.tile_pool(name="ptB", bufs=1))
        opool = ctx.enter_context(tc.tile_pool(name="out", bufs=1))
        psSA = ctx.enter_context(tc.tile_pool(name="psSA", bufs=1, space="PSUM"))
        psSB = ctx.enter_context(tc.tile_pool(name="psSB", bufs=1, space="PSUM"))
        psY = ctx.enter_context(tc.tile_pool(name="psY", bufs=1, space="PSUM"))
        psX = ctx.enter_context(tc.tile_pool(name="psX", bufs=1, space="PSUM"))

        def body(_iv=None):
            # ---- loads ----
            # Chunked so the first V-proj matmul only waits for Wv[:, :512]
            # and xT[:, :256] (~3.5us of DMA) rather than every input. xT
            # goes on the DVE ring: at the reps-loop boundary its WAR (the
            # last pair's K projection) releases late, and on a shared ring
            # it would block the weight reloads queued behind it.
            xTr = const.tile([P, KS, T], bf16, tag="xT", name="xTr")
            Wr = {}
            for w in ("v", "p"):
                Wr[w] = const.tile([P, KS, C], bf16, tag=f"W{w}", name=f"W{w}r")
            Wqk_t = const.tile([P, 2, KS, C], bf16, tag="Wqk", name="Wqkr")
            Wr["q"] = Wqk_t[:, 0]
            Wr["k"] = Wqk_t[:, 1]
            bias_t = const.tile([P, 2 * KS + 2 * C], f32, tag="biases",
                                name="biast")
            bqT = bias_t[:, 0:KS]
            bkT = bias_t[:, KS:2 * KS]
            bvB = bias_t[:, 2 * KS:2 * KS + C]
            bpB = bias_t[:, 2 * KS + C:2 * KS + 2 * C]
            xT_r = xT_d.rearrange("(ks p) t -> p ks t", p=P)
            W_r = {w: W_d[w].rearrange("(ks p) c -> p ks c", p=P)
                   for w in ("v", "p")}
            Wqk_r = Wqk_d.rearrange("(two ks p) c -> p two ks c", p=P, ks=KS)
            nc.sync.dma_start(xTr[:, :, 0:128], xT_r[:, :, 0:128])
            nc.sync.dma_start(Wr["v"][:, :, 0:512], W_r["v"][:, :, 0:512])
            nc.sync.dma_start(Wr["v"][:, :, 512:768], W_r["v"][:, :, 512:768])
            nc.sync.dma_start(bias_t[:], bias_d[:, :])
            nc.gpsimd.dma_start(xTr[:, :, 128:384], xT_r[:, :, 128:384])
            nc.gpsimd.dma_start(xTr[:, :, 384:768], xT_r[:, :, 384:768])
            nc.gpsimd.dma_start(xTr[:, :, 768:1024], xT_r[:, :, 768:1024])
            nc.sync.dma_start(Wqk_t[:], Wqk_r)
            nc.sync.dma_start(Wr["p"][:], W_r["p"])

            ones1 = const.tile([P, 1], f32, tag="ones", name="ones1")
            nc.vector.memset(ones1[:], 1.0)

            # ---- V projection into V_aug (bf16) with ones column ----
            # ones at column 64: att@V's denominator row lands on PSUM
            # partition 64 (32-aligned, so the DVE reciprocal can read it
            # directly from PSUM — no ScalarE staging copy).
            V_aug = const.tile([P, TS, H, D + 1], bf16, tag="Vaug", name="Vaug")
            nc.vector.tensor_copy(V_aug[:, :, :, D:D + 1],
                                  ones1[:].to_broadcast([P, TS, H, 1]))
            for ts_ in range(TS):
                psv = (psY.tile([P, 1024], f32, tag="psyA", name="psv")
                       if ts_ % 2 == 0 else
                       psX.tile([P, 1024], f32, tag="aux", name="psv"))
                # NOTE: matmul groups must stay PSUM-bank-aligned (512
                # fp32): sub-bank column splits create two accumulation
                # groups in one bank, which corrupts silently on HW.
                for k in range(KS):
                    lhsT = xTr[:, k, ts_ * P:(ts_ + 1) * P]
                    nc.tensor.matmul(psv[:, 0:512], lhsT, Wr["v"][:, k, 0:512],
                                     start=(k == 0), stop=(k == KS - 1))
                    nc.tensor.matmul(psv[:, 512:768], lhsT, Wr["v"][:, k, 512:768],
                                     start=(k == 0), stop=(k == KS - 1))
                nc.vector.tensor_tensor(
                    V_aug[:, ts_, :, 0:D],
                    psv[:, 0:768].rearrange("p (h d) -> p h d", h=H),
                    bvB.rearrange("p (h d) -> p h d", h=H), op=ALU.add)

            if phases < 3:
                YTdummy = opool.tile([P, C], f32, tag="ot0", name="ytd")
                nc.vector.memset(YTdummy[:], 0.0)
                nc.sync.dma_start(y_d[0:P, :], YTdummy[:])
                return

            YT = const.tile([P, KS, T], bf16, tag="YTs", name="YT")

            def emit_qk_mms(p, which, ps, kslice):
                w = "q" if which == "Q" else "k"
                for ih in range(2):
                    for k in kslice:
                        nc.tensor.matmul(
                            ps[:, ih * 512:(ih + 1) * 512],
                            Wr[w][:, k, p * P:(p + 1) * P],
                            xTr[:, k, ih * 512:(ih + 1) * 512],
                            start=(k == 0), stop=(k == KS - 1))

            def emit_qk_evac(p, which, ps, split=False):
                bias = bqT if which == "Q" else bkT
                out = qkp.tile([P, T], bf16, tag=f"{which}{p % 2}", name=f"{which}T2")
                # split=True: evacuate cols [0:512] first — the next pair's
                # S^T js 0-3 read only that half (subtile deps), so it can
                # start while the second half still evacuates
                for c0, c1 in ((0, 512), (512, 1024)) if split else ((0, 1024),):
                    nc.vector.tensor_tensor(
                        out[:, c0:c1], ps[:, c0:c1],
                        bias[:, p:p + 1].to_broadcast([P, c1 - c0]),
                        op=ALU.add)
                return out

            def emit_qk_proj(p, which):
                # pair-0 only: SA/SB are idle here, and using them avoids
                # serializing Q-proj -> aux evac -> K-proj -> aux evac
                ps = (psSA.tile([P, 1024], f32, tag="SA", name=f"ps{which}")
                      if which == "Q" else
                      psSB.tile([P, 1024], f32, tag="SB", name=f"ps{which}"))
                emit_qk_mms(p, which, ps, range(KS))
                # split evac: S^T(0,0) writes these regions and can start
                # once cols [0:512] are evacuated
                return emit_qk_evac(p, which, ps, split=True)

            def evac_copy(p, hh, psy):
                """Fast PSUM->SBUF evacuation of the Ytil rows via DVE.
                The denominator row (64) is read by evac_recip directly."""
                tmp = qkp.tile([P, T], f32, tag=f"tmp{hh}{p % 2}", name="tmp")
                nc.vector.tensor_copy(tmp[0:D, :], psy[0:D, :])
                return tmp

            def evac_recip(p, hh, psy):
                """Reciprocal of the denominator row (read straight from PSUM
                partition 0) + GpSimd partition-broadcast across 64 rows; the
                ~1.5us broadcast runs while the PE continues."""
                h = 2 * p + hh
                # staging on DVE, NOT ScalarE: ACT sits on the cycle-
                # critical exp path (exp_A(j) gates S_A(j+1)), and a ~1us
                # copy at the head of its queue delays the whole pair.
                # DVE cross-base copy is legal (both bases 32-aligned).
                dst = qkp.tile([1, T], f32, tag=f"dst{h % 2}", name="dstage")
                nc.vector.tensor_copy(dst[:], psy[D:D + 1, :])
                rcp = qkp.tile([1, T], f32, tag=f"rcp{h % 2}", name="rcp")
                nc.vector.reciprocal_approx_fast(rcp[:], dst[:])
                rb = qkp.tile([D, T], f32, tag=f"rb{h % 2}", name="rb")
                nc.gpsimd.partition_broadcast(rb[:], rcp[:])
                return rb

            def evac_mult(p, hh, tmp, rb):
                b0 = 64 * hh
                nc.vector.tensor_tensor(YT[b0:b0 + 64, p, :], tmp[0:D, :],
                                        rb[:], op=ALU.mult)

            def attv(psy, v_slice, pt, j):
                for ih in range(2):
                    sl = slice(ih * 512, (ih + 1) * 512)
                    nc.tensor.matmul(psy[0:D + 1, sl], v_slice, pt[:, sl],
                                     start=(j == 0), stop=(j == TS - 1))

            # ---- attention pair loop (software-pipelined) ----
            qt_cur = emit_qk_proj(0, "Q")
            kt_cur = emit_qk_proj(0, "K")
            tmpA_prev = tmpB_prev = None
            psyA_prev = None
            qt_nxt = kt_nxt = None
            ptA_tiles = [None] * TS
            ptB_tiles = [None] * TS
            ptB_prev = [None] * TS

            for p in range(npairs):
                last = (p == NP - 1)
                rbA = None
                if psyA_prev is not None:
                    rbA = evac_recip(p - 1, 0, psyA_prev)
                    tmpA_prev = evac_copy(p - 1, 0, psyA_prev)
                psyA = psY.tile([P, 1024], f32, tag="psyA", name="psyA")

                prevB = None
                if p > 0:
                    prevB = psX.tile([P, 1024], f32, tag="aux", name="psyBprev")
                psyB = None

                qps = kps = None
                for j in range(TS):
                    psA = psSA.tile([P, 1024], f32, tag="SA", name="psA")
                    psB = psSB.tile([P, 1024], f32, tag="SB", name="psB")
                    ptA = ptAp.tile([P, 1024], bf16, tag=f"ptA{j}", name="ptA")
                    ptB = ptBp.tile([P, 1024], bf16, tag=f"ptB{j}", name="ptB")

                    # INTERLEAVED A/B emission (A-h0, B-h0, A-h1, B-h1): the
                    # two PE row-groups stream concurrently, so B-h0 starts
                    # while A-h1 streams; A,A,B,B order serializes the same-
                    # group pairs first and delays exp_B by two matmuls
                    for ih in range(2):
                        sl = slice(ih * 512, (ih + 1) * 512)
                        nc.tensor.matmul(psA[:, sl],
                                         kt_cur[0:64, j * P:(j + 1) * P],
                                         qt_cur[0:64, sl],
                                         start=True, stop=True,
                                         tile_position=(0, 0))
                        nc.tensor.matmul(psB[:, sl],
                                         kt_cur[64:128, j * P:(j + 1) * P],
                                         qt_cur[64:128, sl],
                                         start=True, stop=True,
                                         tile_position=(64, 0))
                    if j == TS - 1 and last:
                        # last pair: B's chain is the tail-critical one
                        # (inline attv_B(7) comes first) — emit its exp
                        # first, both split for finer consumer wake-up
                        for c0, c1 in ((0, 512), (512, 1024)):
                            nc.scalar.activation(ptB[:, c0:c1], psB[:, c0:c1],
                                                 AF.Exp, scale=0.125)
                        for c0, c1 in ((0, 512), (512, 1024)):
                            nc.scalar.activation(ptA[:, c0:c1], psA[:, c0:c1],
                                                 AF.Exp, scale=0.125)
                    elif j == TS - 1:
                        # split so the post-loop attv_A(7) (and the next
                        # pair's S^T) wait on a 512-col exp, not the full one
                        nc.scalar.activation(ptA[:, 0:512], psA[:, 0:512],
                                             AF.Exp, scale=0.125)
                        nc.scalar.activation(ptA[:, 512:1024], psA[:, 512:1024],
                                             AF.Exp, scale=0.125)
                        nc.scalar.activation(ptB[:], psB[:], AF.Exp, scale=0.125)
                    else:
                        nc.scalar.activation(ptA[:], psA[:], AF.Exp, scale=0.125)
                        nc.scalar.activation(ptB[:], psB[:], AF.Exp, scale=0.125)
                    ptA_tiles[j] = ptA
                    ptB_tiles[j] = ptB

                    # attV_A one j behind: exp_A(j-1) finished during S(j)
                    if j >= 1:
                        attv(psyA, V_aug[:, j - 1, 2 * p, :], ptA_tiles[j - 1], j - 1)
                    # previous pair's head-B att@V: 3/3/2 chunks on js 0-2,
                    # so its PSUM slot frees early for the next projections
                    if prevB is not None and j < 3:
                        for jj in range(3 * j, min(3 * j + 3, TS)):
                            attv(prevB, V_aug[:, jj, 2 * (p - 1) + 1, :],
                                 ptB_prev[jj], jj)
                        if j == 2:
                            rbB = evac_recip(p - 1, 1, prevB)
                            tmpB_prev = evac_copy(p - 1, 1, prevB)
                    if j == 3 and rbA is not None:
                        evac_mult(p - 1, 0, tmpA_prev, rbA)
                        tmpA_prev = rbA = None
                    if j == 7 and prevB is not None:
                        # j==7 (not 6): keeps the DVE free at j==6 so the kt
                        # evacuation isn't queued behind this mult — YT[p-1]
                        # is only read by the final out-projection anyway
                        evac_mult(p - 1, 1, tmpB_prev, rbB)
                    # next pair's projections on js 3-6
                    if p + 1 < NP:
                        if j == 3:
                            qps = psX.tile([P, 1024], f32, tag="aux", name="psQ")
                            emit_qk_mms(p + 1, "Q", qps, range(0, 3))
                        elif j == 4:
                            emit_qk_mms(p + 1, "Q", qps, range(3, KS))
                            qt_nxt = emit_qk_evac(p + 1, "Q", qps)
                        elif j == 5:
                            kps = psX.tile([P, 1024], f32, tag="aux", name="psK")
                            emit_qk_mms(p + 1, "K", kps, range(0, 3))
                        elif j == 6:
                            emit_qk_mms(p + 1, "K", kps, range(3, KS))
                            kt_nxt = emit_qk_evac(p + 1, "K", kps, split=True)
                    elif last:
                        # pair 5: head-B att@V inline on js 3-7 once the aux
                        # slot frees (accumulation order over j is free)
                        if j == 3:
                            psyB = psX.tile([P, 1024], f32, tag="aux", name="psyB5")
                        if j >= 3:
                            for jj in ([2 * (j - 3), 2 * (j - 3) + 1] if j <= 5
                                       else [j]):
                                attv(psyB, V_aug[:, jj, 2 * p + 1, :],
                                     ptB_tiles[jj], jj)

                attv(psyA, V_aug[:, TS - 1, 2 * p, :], ptA_tiles[TS - 1], TS - 1)
                psyA_prev = psyA
                ptB_prev = list(ptB_tiles)
                qt_cur, kt_cur = qt_nxt, kt_nxt

            # tail: last pair's evacuations, B chain first (its attv(7) is
            # inline and finishes before attv_A(7)). Reciprocals before
            # copies on the DVE so the GpSimd broadcasts launch early; the
            # normalize chain then hides behind the out-projection's k=0..4
            # partial sums on all four freed PSUM regions.
            rbB = tmpB_prev = None
            if psyB is not None:
                rbB = evac_recip(NP - 1, 1, psyB)
            rbA = evac_recip(npairs - 1, 0, psyA_prev)
            if psyB is not None:
                tmpB_prev = evac_copy(NP - 1, 1, psyB)
            tmpA_prev = evac_copy(npairs - 1, 0, psyA_prev)

            if phases < 4:
                # still need the normalize so YT is complete
                evac_mult(npairs - 1, 0, tmpA_prev, rbA)
                if psyB is not None:
                    evac_mult(NP - 1, 1, tmpB_prev, rbB)
                return

            # ---- output projection ----
            # psyA/aux host the EARLY chunks: the next rep's V projection
            # reuses those regions first, so their last readers (bias-adds)
            # must not be the final chunks of this rep
            def po_tile(ts_):
                pool, tag = ((psY, "psyA"), (psX, "aux"),
                             (psSA, "SA"), (psSB, "SB"))[ts_ % 4]
                return pool.tile([P, 1024], f32, tag=tag, name="po")

            def po_mms(po, ts_, kslice):
                for k in kslice:
                    lhsT = YT[:, k, ts_ * P:(ts_ + 1) * P]
                    nc.tensor.matmul(po[:, 0:512], lhsT, Wr["p"][:, k, 0:512],
                                     start=(k == 0), stop=(k == KS - 1))
                    nc.tensor.matmul(po[:, 512:768], lhsT, Wr["p"][:, k, 512:768],
                                     start=(k == 0), stop=(k == KS - 1))

            pos = {}
            for ts_ in (0, 1, 2, 3):
                pos[ts_] = po_tile(ts_)
                po_mms(pos[ts_], ts_, range(KS - 1))

            if psyB is not None:
                evac_mult(NP - 1, 1, tmpB_prev, rbB)
            evac_mult(npairs - 1, 0, tmpA_prev, rbA)

            for ts_ in range(TS):
                if ts_ in pos:
                    po = pos[ts_]
                    po_mms(po, ts_, range(KS - 1, KS))
                else:
                    po = po_tile(ts_)
                    po_mms(po, ts_, range(KS))
                ot = opool.tile([P, C], f32, tag=f"ot{ts_ % 4}", name="ot")
                nc.vector.tensor_tensor(ot[:], po[:, 0:768], bpB, op=ALU.add)
                nc.scalar.dma_start(y_d[ts_ * P:(ts_ + 1) * P, :], ot[:])

        if reps == 1:
            body()
        else:
            import concourse.mybir as _mb
            with tc.For_i(0, reps, 1, hint_engines=tuple(_mb.ALL_ENGINES)):
                body()

    nc.compile()
    return nc


class _Runner:
    """Compile once, run many times on the 8 axon-tunneled cores via PJRT."""

    def __init__(self, nc, n_cores):
        import jax
        import concourse.mybir as mybir
        from jax.sharding import Mesh, PartitionSpec
        from jax.experimental.shard_map import shard_map
        from concourse.bass2jax import (
            _bass_exec_p, install_neuronx_cc_hook, partition_id_tensor)

        install_neuronx_cc_hook()
        self.jax = jax
        self.n_cores = n_cores
        partition_name = nc.partition_id_tensor.name if nc.partition_id_tensor else None
        in_names, out_names, out_avals, zero_outs = [], [], [], []
        for alloc in nc.m.functions[0].allocations:
            if not isinstance(alloc, mybir.MemoryLocationSet):
                continue
            name = alloc.memorylocations[0].name
            if alloc.kind == "ExternalInput":
                if name != partition_name:
                    in_names.append(name)
            elif alloc.kind == "ExternalOutput":
                shape = tuple(alloc.tensor_shape)
                dtype = mybir.dt.np(alloc.dtype)
                out_names.append(name)
                out_avals.append(jax.core.ShapedArray(shape, dtype))
                zero_outs.append(np.zeros(shape, dtype))
        self.in_names, self.out_names = in_names, out_names
        self.zero_outs = zero_outs
        all_in = list(in_names) + list(out_names)
        if partition_name is not None:
            all_in.append(partition_name)

        def _body(*args):
            operands = list(args)
            if partition_name is not None:
                operands.append(partition_id_tensor())
            return tuple(_bass_exec_p.bind(
                *operands, out_avals=tuple(out_avals), in_names=tuple(all_in),
                out_names=tuple(out_names), lowering_input_output_aliases=(),
                sim_require_finite=True, sim_require_nnan=True, nc=nc))

        devices = jax.devices()[:n_cores]
        self.mesh = Mesh(np.asarray(devices), ("core",))
        spec = PartitionSpec("core")
        self.fn = jax.jit(
            shard_map(_body, mesh=self.mesh,
                      in_specs=(spec,) * (len(in_names) + len(out_names)),
                      out_specs=(spec,) * len(out_names), check_rep=False),
            keep_unused=True)

    def stage(self, in_maps):
        import jax
        from jax.sharding import PartitionSpec
        concat = [
            np.concatenate([np.asarray(in_maps[c][n]) for c in range(self.n_cores)], axis=0)
            for n in self.in_names
        ] + [np.concatenate([z] * self.n_cores, axis=0) for z in self.zero_outs]
        sharding = jax.sharding.NamedSharding(self.mesh, PartitionSpec("core"))
        return [jax.device_put(a, sharding) for a in concat]

    def run(self, staged):
        outs = self.fn(*staged)
        self.jax.block_until_ready(outs)
        return outs

    def run_to_maps(self, staged):
        outs = self.run(staged)
        res = []
        for c in range(self.n_cores):
            m = {}
            for i, n in enumerate(self.out_names):
                g = np.asarray(outs[i])
                per = g.shape[0] // self.n_cores
                m[n] = g[c * per:(c + 1) * per]
            res.append(m)
        return res


def get_runner(reps: int = 1, phases: int = 4, npairs: int = NP):
    key = (reps, phases, npairs)
    if key not in _RUNNER_CACHE:
        nc = build_nc(reps, phases, npairs)
        _RUNNER_CACHE[key] = _Runner(nc, N_CORES)
    return _RUNNER_CACHE[key]


def make_in_maps(x, Wq, bq, Wk, bk, Wv, bv, Wp, bp):
    import ml_dtypes
    bf = ml_dtypes.bfloat16
    x = np.asarray(x, dtype=np.float32)
    Wqk = np.ascontiguousarray(
        np.concatenate([np.asarray(Wq, bf), np.asarray(Wk, bf)], axis=0))
    Wv_b = np.asarray(Wv, bf)
    Wp_b = np.asarray(Wp, bf)
    bqT = np.asarray(bq, np.float32).reshape(KS, P).T
    bkT = np.asarray(bk, np.float32).reshape(KS, P).T
    bvB = np.broadcast_to(np.asarray(bv, np.float32), (P, C))
    bpB = np.broadcast_to(np.asarray(bp, np.float32), (P, C))
    biases = np.ascontiguousarray(
        np.concatenate([bqT, bkT, bvB, bpB], axis=1))
    in_maps = []
    for b in range(B):
        in_maps.append({
            "xT": np.ascontiguousarray(x[b].T).astype(bf),
            "Wqk": Wqk, "Wv": Wv_b, "Wp": Wp_b,
            "biases": biases,
        })
    return in_maps


def kernel(x, Wq, bq, Wk, bk, Wv, bv, Wp, bp):
    runner = get_runner(reps=1)
    in_maps = make_in_maps(x, Wq, bq, Wk, bk, Wv, bv, Wp, bp)
    staged = runner.stage(in_maps)
    res = runner.run_to_maps(staged)
    return np.stack([res[b]["y"] for b in range(B)], axis=0)


# revision 51
# speedup vs baseline: 1.0259x; 1.0050x over previous
"""Multi-head attention forward on 8 Trainium2 NeuronCores (Bass/Tile), v3.

Problem: nn_MultiHeadAttention — B=8, T=1024, C=768, H=12, D=64, fp32 in/out.

Sharding: data-parallel over batch — one batch element per core; weights
broadcast (each core loads its own copy). No collectives. Host pre-transposes
x[b] to x^T [C, T] and converts x/W to bf16; fp32 PSUM accumulation keeps the
final rel err ~6.4e-3 (tolerance 2e-2).

Per-core kernel, all matmul operands bf16:
  1. V = x @ Wv -> V_aug [128, T/128, H, 65] bf16 with a ones column per head
     (the ones row of att@V yields the softmax denominator for free).
  2. Per head pair p: Q^T/K^T chunks [128, T] (pair-major: head A on
     partitions 0-63, head B on 64-127) via matmul(lhsT=W[:,co], rhs=xT).
     Pair 0 projects into the idle SA/SB PSUM regions (no aux serialization).
     The mid-loop K projections evacuate cols [0:512] first and the head-B
     normalize mult sits at j=7, so the next pair's S^T never queues behind
     a full-width DVE evacuation at the boundary.
  3. S^T per pair via ROW-TILED matmuls (tile_position (0,0)/(64,0) run the
     two heads concurrently on the PE), SPLIT per head into two separate
     2-bank PSUM regions (SA for head A, SB for head B) with one exp each:
     the PE can start S_A(j+1) as soon as exp_A(j) is done, while exp_B(j)
     still runs (v2's fused [128,2048] region serialized PE<->ACT per j).
     At j=7 the exps are split into 512-col halves (B first on the last
     pair) so pair-boundary/tail consumers wake earlier.
  4. att@V: Ytil[65, :] += V_aug^T @ pt chunks, software-pipelined one j
     behind the exp. Head B's att@V runs early in the NEXT pair's j-loop
     (js 0-2); next pair's Q/K projections on js 3-6; the last pair's B runs
     inline on js 3-7. PSUM: SA 2 + SB 2 + psyA 2 + aux(time-shared) 2 = 8
     banks exactly.
  5. Normalization at Ytil evacuation: denominator row staged via a DVE
     cross-base copy to partition 0 (the custom DVE reciprocal silently
     misreads any other base; ScalarE must NOT do this copy — it sits on
     the cycle-critical exp queue, HW-measured ~7us slower), DVE
     reciprocal, GpSimd partition-broadcast (~1.5us, off the PE), DVE copy
     of the Ytil rows (frees the accumulator banks), DVE multiply into YT.
  6. Tail/output projection: the normalize chain of the last pair hides
     behind out-proj k=0..4 partial-sum matmuls for the first 4 t-chunks on
     all four freed PSUM regions; only the k=5 matmuls wait. 4 rotating ot
     buffers so the last bias-adds don't wait on earlier output DMAs
     (scalar-engine ring).
  Prologue: xT[:, :128] + Wv load first (sync ring) so the first V-proj
  matmul issues at ~4us; remaining xT t-chunks stream on the GpSimd SWDGE
  ring concurrently with the other weights (their reps-loop WAR releases
  late and would block the sync ring). Wp loads into its own buffer.

HW notes (learned the hard way, enforced by CoreSim/BIR but not TimelineSim):
  - PSUM matmul accumulation groups must be bank-aligned (512 fp32); a
    matmul may not cross a bank boundary.
  - Engine APs touching PSUM need 32-aligned partition bases.
  - Finer-grained pipelining (4 exps/j with alternating 1-bank S tiles)
    measured SLOWER on HW: per-hop semaphore latency dominates, so fewer,
    bigger sync units win.
  - Attention steady state is bound by the serial cycle S_A(j) -> exp_A(j)
    -> S_A(j+1) (~3.4us/j at exp=764ns); deeper S buffering needs 12 PSUM
    banks (only 8 exist). Keep the ACT queue clean: exps only.
"""
import numpy as np

B, T, C = 8, 1024, 768
H, D = 12, 64
P = 128
KS = C // P          # 6 contraction subtiles
TS = T // P          # 8 t subtiles
NP = H // 2          # 6 head pairs
N_CORES = 8

_RUNNER_CACHE = {}


def build_nc(reps: int = 1, phases: int = 4, npairs: int = NP):
    import concourse.bacc as bacc
    import concourse.mybir as mybir
    import concourse.tile as tile
    from contextlib import ExitStack

    f32 = mybir.dt.float32
    bf16 = mybir.dt.bfloat16
    AF = mybir.ActivationFunctionType
    ALU = mybir.AluOpType

    nc = bacc.Bacc(num_devices=N_CORES)

    xT_d = nc.dram_tensor("xT", [C, T], bf16, kind="ExternalInput")
    W_d = {w: nc.dram_tensor(f"W{w}", [C, C], bf16, kind="ExternalInput")
           for w in ("v", "p")}
    # Wq|Wk packed into one tensor, all four biases into another: fewer
    # HWDGE descriptors (~625ns issue + latency gap each) on the sync ring
    Wqk_d = nc.dram_tensor("Wqk", [2 * C, C], bf16, kind="ExternalInput")
    bias_d = nc.dram_tensor("biases", [P, 2 * KS + 2 * C], f32,
                            kind="ExternalInput")
    y_d = nc.dram_tensor("y", [T, C], f32, kind="ExternalOutput")

    with tile.TileContext(nc) as tc, ExitStack() as ctx:
        const = ctx.enter_context(tc.tile_pool(name="const", bufs=1))
        qkp = ctx.enter_context(tc.tile_pool(name="qk", bufs=1))
        ptAp = ctx.enter_context(tc.tile_pool(name="ptA", bufs=1))
        ptBp = ctx.enter_context(tc.tile_pool(name="ptB", bufs=1))
        opool = ctx.enter_context(tc.tile_pool(name="out", bufs=1))
        psSA = ctx.enter_context(tc.tile_pool(name="psSA", bufs=1, space="PSUM"))
        psSB = ctx.enter_context(tc.tile_pool(name="psSB", bufs=1, space="PSUM"))
        psY = ctx.enter_context(tc.tile_pool(name="psY", bufs=1, space="PSUM"))
        psX = ctx.enter_context(tc.tile_pool(name="psX", bufs=1, space="PSUM"))

        def body(_iv=None):
            # ---- loads ----
            # Chunked so the first V-proj matmul only waits for Wv[:, :512]
            # and xT[:, :256] (~3.5us of DMA) rather than every input. xT
            # goes on the DVE ring: at the reps-loop boundary its WAR (the
            # last pair's K projection) releases late, and on a shared ring
            # it would block the weight reloads queued behind it.
            xTr = const.tile([P, KS, T], bf16, tag="xT", name="xTr")
            Wr = {}
            for w in ("v", "p"):
                Wr[w] = const.tile([P, KS, C], bf16, tag=f"W{w}", name=f"W{w}r")
            Wqk_t = const.tile([P, 2, KS, C], bf16, tag="Wqk", name="Wqkr")
            Wr["q"] = Wqk_t[:, 0]
            Wr["k"] = Wqk_t[:, 1]
            bias_t = const.tile([P, 2 * KS + 2 * C], f32, tag="biases",
                                name="biast")
            bqT = bias_t[:, 0:KS]
            bkT = bias_t[:, KS:2 * KS]
            bvB = bias_t[:, 2 * KS:2 * KS + C]
            bpB = bias_t[:, 2 * KS + C:2 * KS + 2 * C]
            xT_r = xT_d.rearrange("(ks p) t -> p ks t", p=P)
            W_r = {w: W_d[w].rearrange("(ks p) c -> p ks c", p=P)
                   for w in ("v", "p")}
            Wqk_r = Wqk_d.rearrange("(two ks p) c -> p two ks c", p=P, ks=KS)
            nc.sync.dma_start(xTr[:, :, 0:128], xT_r[:, :, 0:128])
            nc.sync.dma_start(Wr["v"][:, :, 0:512], W_r["v"][:, :, 0:512])
            nc.sync.dma_start(Wr["v"][:, :, 512:768], W_r["v"][:, :, 512:768])
            nc.sync.dma_start(bias_t[:], bias_d[:, :])
            nc.gpsimd.dma_start(xTr[:, :, 128:384], xT_r[:, :, 128:384])
            nc.gpsimd.dma_start(xTr[:, :, 384:768], xT_r[:, :, 384:768])
            nc.gpsimd.dma_start(xTr[:, :, 768:1024], xT_r[:, :, 768:1024])
            nc.sync.dma_start(Wqk_t[:], Wqk_r)
            nc.sync.dma_start(Wr["p"][:], W_r["p"])

            ones1 = const.tile([P, 1], f32, tag="ones", name="ones1")
            nc.vector.memset(ones1[:], 1.0)

            # ---- V projection into V_aug (bf16) with ones column ----
            # ones at column 64: att@V's denominator row lands on PSUM
            # partition 64 (32-aligned, so the DVE reciprocal can read it
            # directly from PSUM — no ScalarE staging copy).
            V_aug = const.tile([P, TS, H, D + 1], bf16, tag="Vaug", name="Vaug")
            nc.vector.tensor_copy(V_aug[:, :, :, D:D + 1],
                                  ones1[:].to_broadcast([P, TS, H, 1]))
            for ts_ in range(TS):
                psv = (psY.tile([P, 1024], f32, tag="psyA", name="psv")
                       if ts_ % 2 == 0 else
                       psX.tile([P, 1024], f32, tag="aux", name="psv"))
                # NOTE: matmul groups must stay PSUM-bank-aligned (512
                # fp32): sub-bank column splits create two accumulation
                # groups in one bank, which corrupts silently on HW.
                for k in range(KS):
                    lhsT = xTr[:, k, ts_ * P:(ts_ + 1) * P]
                    nc.tensor.matmul(psv[:, 0:512], lhsT, Wr["v"][:, k, 0:512],
                                     start=(k == 0), stop=(k == KS - 1))
                    nc.tensor.matmul(psv[:, 512:768], lhsT, Wr["v"][:, k, 512:768],
                                     start=(k == 0), stop=(k == KS - 1))
                nc.vector.tensor_tensor(
                    V_aug[:, ts_, :, 0:D],
                    psv[:, 0:768].rearrange("p (h d) -> p h d", h=H),
                    bvB.rearrange("p (h d) -> p h d", h=H), op=ALU.add)

            if phases < 3:
                YTdummy = opool.tile([P, C], f32, tag="ot0", name="ytd")
                nc.vector.memset(YTdummy[:], 0.0)
                nc.sync.dma_start(y_d[0:P, :], YTdummy[:])
                return

            YT = const.tile([P, KS, T], bf16, tag="YTs", name="YT")

            def emit_qk_mms(p, which, ps, kslice):
                w = "q" if which == "Q" else "k"
                for ih in range(2):
                    for k in kslice:
                        nc.tensor.matmul(
                            ps[:, ih * 512:(ih + 1) * 512],
                            Wr[w][:, k, p * P:(p + 1) * P],
                            xTr[:, k, ih * 512:(ih + 1) * 512],
                            start=(k == 0), stop=(k == KS - 1))

            def emit_qk_evac(p, which, ps, split=False):
                bias = bqT if which == "Q" else bkT
                out = qkp.tile([P, T], bf16, tag=f"{which}{p % 2}", name=f"{which}T2")
                # split=True: evacuate cols [0:512] first — the next pair's
                # S^T js 0-3 read only that half (subtile deps), so it can
                # start while the second half still evacuates
                for c0, c1 in ((0, 512), (512, 1024)) if split else ((0, 1024),):
                    nc.vector.tensor_tensor(
                        out[:, c0:c1], ps[:, c0:c1],
                        bias[:, p:p + 1].to_broadcast([P, c1 - c0]),
                        op=ALU.add)
                return out

            def emit_qk_proj(p, which):
                # pair-0 only: SA/SB are idle here, and using them avoids
                # serializing Q-proj -> aux evac -> K-proj -> aux evac
                ps = (psSA.tile([P, 1024], f32, tag="SA", name=f"ps{which}")
                      if which == "Q" else
                      psSB.tile([P, 1024], f32, tag="SB", name=f"ps{which}"))
                emit_qk_mms(p, which, ps, range(KS))
                # split evac: S^T(0,0) writes these regions and can start
                # once cols [0:512] are evacuated
                return emit_qk_evac(p, which, ps, split=True)

            def evac_copy(p, hh, psy):
                """Fast PSUM->SBUF evacuation of the Ytil rows via DVE.
                The denominator row (64) is read by evac_recip directly."""
                tmp = qkp.tile([P, T], f32, tag=f"tmp{hh}{p % 2}", name="tmp")
                nc.vector.tensor_copy(tmp[0:D, :], psy[0:D, :])
                return tmp

            def evac_recip(p, hh, psy):
                """Reciprocal of the denominator row (read straight from PSUM
                partition 0) + GpSimd partition-broadcast across 64 rows; the
                ~1.5us broadcast runs while the PE continues."""
                h = 2 * p + hh
                # staging on DVE, NOT ScalarE: ACT sits on the cycle-
                # critical exp path (exp_A(j) gates S_A(j+1)), and a ~1us
                # copy at the head of its queue delays the whole pair.
                # DVE cross-base copy is legal (both bases 32-aligned).
                dst = qkp.tile([1, T], f32, tag=f"dst{h % 2}", name="dstage")
                nc.vector.tensor_copy(dst[:], psy[D:D + 1, :])
                rcp = qkp.tile([1, T], f32, tag=f"rcp{h % 2}", name="rcp")
                nc.vector.reciprocal_approx_fast(rcp[:], dst[:])
                rb = qkp.tile([D, T], f32, tag=f"rb{h % 2}", name="rb")
                nc.gpsimd.partition_broadcast(rb[:], rcp[:])
                return rb

            def evac_mult(p, hh, tmp, rb):
                b0 = 64 * hh
                nc.vector.tensor_tensor(YT[b0:b0 + 64, p, :], tmp[0:D, :],
                                        rb[:], op=ALU.mult)

            def attv(psy, v_slice, pt, j):
                for ih in range(2):
                    sl = slice(ih * 512, (ih + 1) * 512)
                    nc.tensor.matmul(psy[0:D + 1, sl], v_slice, pt[:, sl],
                                     start=(j == 0), stop=(j == TS - 1))

            # ---- attention pair loop (software-pipelined) ----
            qt_cur = emit_qk_proj(0, "Q")
            kt_cur = emit_qk_proj(0, "K")
            tmpA_prev = tmpB_prev = None
            psyA_prev = None
            qt_nxt = kt_nxt = None
            ptA_tiles = [None] * TS
            ptB_tiles = [None] * TS
            ptB_prev = [None] * TS

            for p in range(npairs):
                last = (p == NP - 1)
                rbA = None
                if psyA_prev is not None:
                    rbA = evac_recip(p - 1, 0, psyA_prev)
                    tmpA_prev = evac_copy(p - 1, 0, psyA_prev)
                psyA = psY.tile([P, 1024], f32, tag="psyA", name="psyA")

                prevB = None
                if p > 0:
                    prevB = psX.tile([P, 1024], f32, tag="aux", name="psyBprev")
                psyB = None

                qps = kps = None
                for j in range(TS):
                    psA = psSA.tile([P, 1024], f32, tag="SA", name="psA")
                    psB = psSB.tile([P, 1024], f32, tag="SB", name="psB")
                    ptA = ptAp.tile([P, 1024], bf16, tag=f"ptA{j}", name="ptA")
                    ptB = ptBp.tile([P, 1024], bf16, tag=f"ptB{j}", name="ptB")

                    # INTERLEAVED A/B emission (A-h0, B-h0, A-h1, B-h1): the
                    # two PE row-groups stream concurrently, so B-h0 starts
                    # while A-h1 streams; A,A,B,B order serializes the same-
                    # group pairs first and delays exp_B by two matmuls
                    for ih in range(2):
                        sl = slice(ih * 512, (ih + 1) * 512)
                        nc.tensor.matmul(psA[:, sl],
                                         kt_cur[0:64, j * P:(j + 1) * P],
                                         qt_cur[0:64, sl],
                                         start=True, stop=True,
                                         tile_position=(0, 0))
                        nc.tensor.matmul(psB[:, sl],
                                         kt_cur[64:128, j * P:(j + 1) * P],
                                         qt_cur[64:128, sl],
                                         start=True, stop=True,
                                         tile_position=(64, 0))
                    if j == TS - 1 and last:
                        # last pair: B's chain is the tail-critical one
                        # (inline attv_B(7) comes first) — emit its exp
                        # first, both split for finer consumer wake-up
                        for c0, c1 in ((0, 512), (512, 1024)):
                            nc.scalar.activation(ptB[:, c0:c1], psB[:, c0:c1],
                                                 AF.Exp, scale=0.125)
                        for c0, c1 in ((0, 512), (512, 1024)):
                            nc.scalar.activation(ptA[:, c0:c1], psA[:, c0:c1],
                                                 AF.Exp, scale=0.125)
                    elif j == TS - 1:
                        # split so the post-loop attv_A(7) (and the next
                        # pair's S^T) wait on a 512-col exp, not the full one
                        nc.scalar.activation(ptA[:, 0:512], psA[:, 0:512],
                                             AF.Exp, scale=0.125)
                        nc.scalar.activation(ptA[:, 512:1024], psA[:, 512:1024],
                                             AF.Exp, scale=0.125)
                        nc.scalar.activation(ptB[:], psB[:], AF.Exp, scale=0.125)
                    else:
                        nc.scalar.activation(ptA[:], psA[:], AF.Exp, scale=0.125)
                        nc.scalar.activation(ptB[:], psB[:], AF.Exp, scale=0.125)
                    ptA_tiles[j] = ptA
                    ptB_tiles[j] = ptB

                    # attV_A one j behind: exp_A(j-1) finished during S(j)
                    if j >= 1:
                        attv(psyA, V_aug[:, j - 1, 2 * p, :], ptA_tiles[j - 1], j - 1)
                    # previous pair's head-B att@V: 3/3/2 chunks on js 0-2,
                    # so its PSUM slot frees early for the next projections
                    if prevB is not None and j < 3:
                        for jj in range(3 * j, min(3 * j + 3, TS)):
                            attv(prevB, V_aug[:, jj, 2 * (p - 1) + 1, :],
                                 ptB_prev[jj], jj)
                        if j == 2:
                            rbB = evac_recip(p - 1, 1, prevB)
                            tmpB_prev = evac_copy(p - 1, 1, prevB)
                    if j == 3 and rbA is not None:
                        evac_mult(p - 1, 0, tmpA_prev, rbA)
                        tmpA_prev = rbA = None
                    if j == 7 and prevB is not None:
                        # j==7 (not 6): keeps the DVE free at j==6 so the kt
                        # evacuation isn't queued behind this mult — YT[p-1]
                        # is only read by the final out-projection anyway
                        evac_mult(p - 1, 1, tmpB_prev, rbB)
                    # next pair's projections on js 3-6
                    if p + 1 < NP:
                        if j == 3:
                            qps = psX.tile([P, 1024], f32, tag="aux", name="psQ")
                            emit_qk_mms(p + 1, "Q", qps, range(0, 3))
                        elif j == 4:
                            emit_qk_mms(p + 1, "Q", qps, range(3, KS))
                            qt_nxt = emit_qk_evac(p + 1, "Q", qps)
                        elif j == 5:
                            kps = psX.tile([P, 1024], f32, tag="aux", name="psK")
                            emit_qk_mms(p + 1, "K", kps, range(0, 3))
                        elif j == 6:
                            emit_qk_mms(p + 1, "K", kps, range(3, KS))
                            kt_nxt = emit_qk_evac(p + 1, "K", kps, split=True)
                    elif last:
                        # pair 5: head-B att@V inline on js 3-7 once the aux
                        # slot frees (accumulation order over j is free)
                        if j == 3:
                            psyB = psX.tile([P, 1024], f32, tag="aux", name="psyB5")
                        if j >= 3:
                            for jj in ([2 * (j - 3), 2 * (j - 3) + 1] if j <= 5
                                       else [j]):
                                attv(psyB, V_aug[:, jj, 2 * p + 1, :],
                                     ptB_tiles[jj], jj)

                attv(psyA, V_aug[:, TS - 1, 2 * p, :], ptA_tiles[TS - 1], TS - 1)
                psyA_prev = psyA
                ptB_prev = list(ptB_tiles)
                qt_cur, kt_cur = qt_nxt, kt_nxt

            # tail: last pair's evacuations, B chain first (its attv(7) is
            # inline and finishes before attv_A(7)). Reciprocals before
            # copies on the DVE so the GpSimd broadcasts launch early; the
            # normalize chain then hides behind the out-projection's k=0..4
            # partial sums on all four freed PSUM regions.
            rbB = tmpB_prev = None
            if psyB is not None:
                rbB = evac_recip(NP - 1, 1, psyB)
            rbA = evac_recip(npairs - 1, 0, psyA_prev)
            if psyB is not None:
                tmpB_prev = evac_copy(NP - 1, 1, psyB)
            tmpA_prev = evac_copy(npairs - 1, 0, psyA_prev)

            if phases < 4:
                # still need the normalize so YT is complete
                evac_mult(npairs - 1, 0, tmpA_prev, rbA)
                if psyB is not None:
                    evac_mult(NP - 1, 1, tmpB_prev, rbB)
                return

            # ---- output projection ----
            # psyA/aux host the EARLY chunks: the next rep's V projection
            # reuses those regions first, so their last readers (bias-adds)
            # must not be the final chunks of this rep
            def po_tile(ts_):
                pool, tag = ((psY, "psyA"), (psX, "aux"),
                             (psSA, "SA"), (psSB, "SB"))[ts_ % 4]
                return pool.tile([P, 1024], f32, tag=tag, name="po")

            def po_mms(po, ts_, kslice):
                for k in kslice:
                    lhsT = YT[:, k, ts_ * P:(ts_ + 1) * P]
                    nc.tensor.matmul(po[:, 0:512], lhsT, Wr["p"][:, k, 0:512],
                                     start=(k == 0), stop=(k == KS - 1))
                    nc.tensor.matmul(po[:, 512:768], lhsT, Wr["p"][:, k, 512:768],
                                     start=(k == 0), stop=(k == KS - 1))

            pos = {}
            for ts_ in (0, 1, 2, 3):
                pos[ts_] = po_tile(ts_)
                po_mms(pos[ts_], ts_, range(KS - 1))

            if psyB is not None:
                evac_mult(NP - 1, 1, tmpB_prev, rbB)
            evac_mult(npairs - 1, 0, tmpA_prev, rbA)

            for ts_ in range(TS):
                if ts_ in pos:
                    po = pos[ts_]
                    po_mms(po, ts_, range(KS - 1, KS))
                else:
                    po = po_tile(ts_)
                    po_mms(po, ts_, range(KS))
                ot = opool.tile([P, C], f32, tag=f"ot{ts_ % 4}", name="ot")
                nc.vector.tensor_tensor(ot[:], po[:, 0:768], bpB, op=ALU.add)
                nc.scalar.dma_start(y_d[ts_ * P:(ts_ + 1) * P, :], ot[:])

        if reps == 1:
            body()
        else:
            import concourse.mybir as _mb
            with tc.For_i(0, reps, 1, hint_engines=tuple(_mb.ALL_ENGINES)):
                body()

    nc.compile()
    return nc


class _Runner:
    """Compile once, run many times on the 8 axon-tunneled cores via PJRT."""

    def __init__(self, nc, n_cores):
        import jax
        import concourse.mybir as mybir
        from jax.sharding import Mesh, PartitionSpec
        from jax.experimental.shard_map import shard_map
        from concourse.bass2jax import (
            _bass_exec_p, install_neuronx_cc_hook, partition_id_tensor)

        install_neuronx_cc_hook()
        self.jax = jax
        self.n_cores = n_cores
        partition_name = nc.partition_id_tensor.name if nc.partition_id_tensor else None
        in_names, out_names, out_avals, zero_outs = [], [], [], []
        for alloc in nc.m.functions[0].allocations:
            if not isinstance(alloc, mybir.MemoryLocationSet):
                continue
            name = alloc.memorylocations[0].name
            if alloc.kind == "ExternalInput":
                if name != partition_name:
                    in_names.append(name)
            elif alloc.kind == "ExternalOutput":
                shape = tuple(alloc.tensor_shape)
                dtype = mybir.dt.np(alloc.dtype)
                out_names.append(name)
                out_avals.append(jax.core.ShapedArray(shape, dtype))
                zero_outs.append(np.zeros(shape, dtype))
        self.in_names, self.out_names = in_names, out_names
        self.zero_outs = zero_outs
        all_in = list(in_names) + list(out_names)
        if partition_name is not None:
            all_in.append(partition_name)

        def _body(*args):
            operands = list(args)
            if partition_name is not None:
                operands.append(partition_id_tensor())
            return tuple(_bass_exec_p.bind(
                *operands, out_avals=tuple(out_avals), in_names=tuple(all_in),
                out_names=tuple(out_names), lowering_input_output_aliases=(),
                sim_require_finite=True, sim_require_nnan=True, nc=nc))

        devices = jax.devices()[:n_cores]
        self.mesh = Mesh(np.asarray(devices), ("core",))
        spec = PartitionSpec("core")
        self.fn = jax.jit(
            shard_map(_body, mesh=self.mesh,
                      in_specs=(spec,) * (len(in_names) + len(out_names)),
                      out_specs=(spec,) * len(out_names), check_rep=False),
            keep_unused=True)

    def stage(self, in_maps):
        import jax
        from jax.sharding import PartitionSpec
        concat = [
            np.concatenate([np.asarray(in_maps[c][n]) for c in range(self.n_cores)], axis=0)
            for n in self.in_names
        ] + [np.concatenate([z] * self.n_cores, axis=0) for z in self.zero_outs]
        sharding = jax.sharding.NamedSharding(self.mesh, PartitionSpec("core"))
        return [jax.device_put(a, sharding) for a in concat]

    def run(self, staged):
        outs = self.fn(*staged)
        self.jax.block_until_ready(outs)
        return outs

    def run_to_maps(self, staged):
        outs = self.run(staged)
        res = []
        for c in range(self.n_cores):
            m = {}
            for i, n in enumerate(self.out_names):
                g = np.asarray(outs[i])
                per = g.shape[0] // self.n_cores
                m[n] = g[c * per:(c + 1) * per]
            res.append(m)
        return res


def get_runner(reps: int = 1, phases: int = 4, npairs: int = NP):
    key = (reps, phases, npairs)
    if key not in _RUNNER_CACHE:
        nc = build_nc(reps, phases, npairs)
        _RUNNER_CACHE[key] = _Runner(nc, N_CORES)
    return _RUNNER_CACHE[key]


def make_in_maps(x, Wq, bq, Wk, bk, Wv, bv, Wp, bp):
    import ml_dtypes
    bf = ml_dtypes.bfloat16
    x = np.asarray(x, dtype=np.float32)
    Wqk = np.ascontiguousarray(
        np.concatenate([np.asarray(Wq, bf), np.asarray(Wk, bf)], axis=0))
    Wv_b = np.asarray(Wv, bf)
    Wp_b = np.asarray(Wp, bf)
    bqT = np.asarray(bq, np.float32).reshape(KS, P).T
    bkT = np.asarray(bk, np.float32).reshape(KS, P).T
    bvB = np.broadcast_to(np.asarray(bv, np.float32), (P, C))
    bpB = np.broadcast_to(np.asarray(bp, np.float32), (P, C))
    biases = np.ascontiguousarray(
        np.concatenate([bqT, bkT, bvB, bpB], axis=1))
    in_maps = []
    for b in range(B):
        in_maps.append({
            "xT": np.ascontiguousarray(x[b].T).astype(bf),
            "Wqk": Wqk, "Wv": Wv_b, "Wp": Wp_b,
            "biases": biases,
        })
    return in_maps


def kernel(x, Wq, bq, Wk, bk, Wv, bv, Wp, bp):
    runner = get_runner(reps=1)
    in_maps = make_in_maps(x, Wq, bq, Wk, bk, Wv, bv, Wp, bp)
    staged = runner.stage(in_maps)
    res = runner.run_to_maps(staged)
    return np.stack([res[b]["y"] for b in range(B)], axis=0)
